# revision 33
# baseline (speedup 1.0000x reference)
"""DiagonalLSTM Trainium2 kernel — band-restricted scan.

Sharding: data-parallel over batch B=16 across 8 cores (2 batch elems/core).
Per-core layout: partitions = 128-wide HID gate chunks, free dim = (b, j)
where j indexes the LIVE DIAGONAL BAND rows [lo..hi], lo = max(0, t-63),
hi = min(t, 63).

Key reduction vs the full-width scan: rows r > t ("virgin" rows, zero x so
far) all share one state vector v_t that depends only on t, so they are not
computed on-device at all.  A host-precomputed fp64 table of v_t (h and c,
device 2x convention) seeds row t+1 each step via two 1-col gpsimd copies.
Per-step matmul/ACT/DVE free size drops from avg 96 to avg 64.5 columns and
the x-side input is pre-packed band-only (xsk [64, 8192] vs [64, 16256]).

Per scan step t (127 steps), each of the 5 gate chunks accumulates in PSUM:
    wis_chunk @ x_band   (K=64, packed band cols, opens the group)
  + w0_chunk @ h_prev    row-shifted (skipped at t=0)
  + w1_chunk @ h_prev
All scan matmuls fp32: the scan dynamics chaotically amplify per-step input
rounding (measured: fp32r inputs -> rel err 1.4), so only the feed-forward
residual matmul uses fp32r (4x faster, error enters once, ~1e-4).

Sigmoid gates computed as 0.5*(1+tanh(x/2)) via pre-halved weights; ONE tanh
activation per gate chunk (fires as soon as its PSUM bank closes, keeping the
serial chain short).  State convention: h_cur holds 2h, c_cur holds 2c; res
accumulates 2*(h+residual) and the host halves the output.
"""

import numpy as np

import concourse.bass as bass
import concourse.mybir as mybir
from concourse import bacc
from concourse import tile
from concourse.bass_utils import run_bass_kernel_spmd

B, C, H, W = 16, 64, 64, 64
HID = 128
SW = H + W - 1  # 127
NCORES = 8
BL = B // NCORES  # 2
NBH = BL * H       # 128 state cols (b, r)
NRES = BL * H * W  # 8192 output cols

F32 = mybir.dt.float32
F32R = mybir.dt.float32r
AF = mybir.ActivationFunctionType
ALU = mybir.AluOpType

# band geometry per step (shared host/device)
_LO = [max(0, t - (W - 1)) for t in range(SW)]
_HI = [min(t, H - 1) for t in range(SW)]
_M = [hi - lo + 1 for lo, hi in zip(_LO, _HI)]
_BASE = np.concatenate([[0], np.cumsum([BL * m for m in _M])]).astype(int)
XC = int(_BASE[-1])  # 8192

# scan chunk emission order (gate chunk index k): fl, fu, i, g, o
KORD = (1, 2, 3, 4, 0)


def _raw(t, off, dims):
    """Raw AP on tile t: keep its partition pair, custom free dims."""
    return bass.AP(t.tensor, t.offset + off, [list(t.ap[0])] + [list(d) for d in dims])


def build_program():
    nc = bacc.Bacc(None, target_bir_lowering=False)

    xsk_d = nc.dram_tensor("xsk", [C + 1, XC], F32, kind="ExternalInput")
    xres_d = nc.dram_tensor("xres", [C + 1, NRES], F32R, kind="ExternalInput")
    wtap_d = nc.dram_tensor("wtap", [HID, 2 * 5 * HID], F32, kind="ExternalInput")
    wis_d = nc.dram_tensor("wis", [C + 1, 5 * HID], F32, kind="ExternalInput")
    wres_d = nc.dram_tensor("wres", [C + 1, HID], F32R, kind="ExternalInput")
    hv_d = nc.dram_tensor("hv", [HID, H - 1], F32, kind="ExternalInput")
    cv_d = nc.dram_tensor("cv", [HID, H - 1], F32, kind="ExternalInput")
    out_d = nc.dram_tensor("out", [HID, NRES], F32, kind="ExternalOutput")

    with tile.TileContext(nc) as tc:
        with (
            tc.tile_pool(name="const", bufs=1) as const,
            tc.tile_pool(name="state", bufs=3) as state,
            tc.tile_pool(name="tmp", bufs=3) as tmp,
            tc.tile_pool(name="gpsumA", bufs=3, space="PSUM") as gpsumA,
            tc.tile_pool(name="gpsumO", bufs=2, space="PSUM") as gpsumO,
        ):
            xsk = const.tile([C + 1, XC], F32)
            xres = const.tile([C + 1, NRES], F32R)
            wtap = const.tile([HID, 2 * 5 * HID], F32)
            wis = const.tile([C + 1, 5 * HID], F32)
            wres = const.tile([C + 1, HID], F32R)
            hv = const.tile([HID, H - 1], F32)
            cv = const.tile([HID, H - 1], F32)
            res = const.tile([HID, NRES], F32)

            nc.sync.dma_start(out=wis, in_=wis_d[:])
            nc.sync.dma_start(out=wtap, in_=wtap_d[:])
            nc.sync.dma_start(out=hv, in_=hv_d[:])
            nc.sync.dma_start(out=cv, in_=cv_d[:])
            nc.sync.dma_start(out=wres, in_=wres_d[:])
            steps_cut = [0, 4, 12, 24, 36, 48, 64, 80, 100, SW]
            for a, b in zip(steps_cut[:-1], steps_cut[1:]):
                lo_e, hi_e = int(_BASE[a]), int(_BASE[b])
                nc.sync.dma_start(out=xsk[:, lo_e:hi_e], in_=xsk_d[:, lo_e:hi_e])
            nc.sync.dma_start(out=xres, in_=xres_d[:])

            def pbankA():
                ps = gpsumA.tile([HID, 1024], F32, tag="A")
                return ps

            def pbankO():
                ps = gpsumO.tile([HID, 512], F32, tag="O")
                return ps

            # ---- scan state: (b, r) layout [HID, 128] ----
            h_cur = state.tile([HID, NBH], F32, tag="h")
            c_cur = state.tile([HID, NBH], F32, tag="c")
            nc.vector.memzero(h_cur)
            nc.vector.memzero(c_cur)

            def B3(ap, a, b):
                """(b, r) state view, rows [a..b) of each batch block."""
                return ap.rearrange("p (b r) -> p b r", b=BL)[:, :, a:b]

            # gate chunk placement: A0 = {fl, fu}, A1 = {i, g}, O = {o};
            # each chunk in its own PSUM bank (512-col offsets), pairs share
            # a 2-bank tile so ONE activation covers both chunks.
            def slots(tiles):
                a0, a1, po = tiles
                return ((a0, 0), (a0, 512), (a1, 0), (a1, 512), (po, 0))

            def xmm(t):
                """i_s matmuls for step t: packed band cols (opens groups).
                K=65: the ones row of xsk adds the per-gate bias."""
                b0, n = int(_BASE[t]), BL * _M[t]
                tiles = (pbankA(), pbankA(), pbankO())
                for idx, k in enumerate(KORD):
                    pk, off = slots(tiles)[idx]
                    nc.tensor.matmul(
                        _raw(pk, off, [[1, n]]),
                        wis[:, k * HID:(k + 1) * HID],
                        xsk[:, b0:b0 + n],
                        start=True, stop=False,
                    )
                return tiles

            pcur = xmm(0)

            for t in range(SW):
                lo, hi, m = _LO[t], _HI[t], _M[t]
                n = BL * m
                s0 = max(lo, 1)
                mq = hi - s0 + 1  # rows with a defined (r-1) neighbor

                th = tmp.tile([HID, 5 * HID], F32, tag="th")
                for idx, k in enumerate(KORD):
                    pk, off = slots(pcur)[idx]
                    w0c = wtap[:, k * HID:(k + 1) * HID]
                    w1c = wtap[:, 5 * HID + k * HID:5 * HID + (k + 1) * HID]
                    if mq > 0:
                        nc.tensor.matmul(
                            _raw(pk, off + s0 - lo, [[m, BL], [1, mq]]),
                            w0c,
                            B3(h_cur, s0 - 1, hi),
                            start=False, stop=False,
                        )
                    nc.tensor.matmul(
                        _raw(pk, off, [[m, BL], [1, m]]),
                        w1c,
                        B3(h_cur, lo, hi + 1),
                        start=False, stop=True,
                    )
                    # paired tanh: fires when both banks of the pair close
                    if idx in (1, 3):
                        nc.scalar.activation(
                            _raw(th, (idx - 1) * HID, [[HID, 2], [1, n]]),
                            _raw(pk, 0, [[512, 2], [1, n]]),
                            AF.Tanh,
                        )
                    elif idx == 4:
                        nc.scalar.activation(
                            _raw(th, 4 * HID, [[1, n]]),
                            _raw(pk, 0, [[1, n]]),
                            AF.Tanh,
                        )

                # prefetch next step's x-side matmuls while ACT/DVE run
                if t + 1 < SW:
                    pcur = xmm(t + 1)

                # P = (t_fl+1)*C2 ; P += (t_fu+1)*C2sh (rows >= s0);
                # C2' = 0.5*P + (t_i+1)*g  on band rows
                p = tmp.tile([HID, NBH], F32, tag="p")
                nc.vector.scalar_tensor_tensor(
                    _raw(p, 0, [[m, BL], [1, m]]),
                    _raw(th, 0 * HID, [[m, BL], [1, m]]),
                    1.0, B3(c_cur, lo, hi + 1), op0=ALU.add, op1=ALU.mult,
                )
                if mq > 0:
                    q = tmp.tile([HID, NBH], F32, tag="q")
                    nc.vector.scalar_tensor_tensor(
                        _raw(q, s0 - lo, [[m, BL], [1, mq]]),
                        _raw(th, 1 * HID + (s0 - lo), [[m, BL], [1, mq]]),
                        1.0, B3(c_cur, s0 - 1, hi), op0=ALU.add, op1=ALU.mult,
                    )
                    nc.vector.tensor_add(
                        _raw(p, s0 - lo, [[m, BL], [1, mq]]),
                        _raw(p, s0 - lo, [[m, BL], [1, mq]]),
                        _raw(q, s0 - lo, [[m, BL], [1, mq]]),
                    )
                r_t = tmp.tile([HID, NBH], F32, tag="r_t")
                nc.vector.scalar_tensor_tensor(
                    _raw(r_t, 0, [[1, n]]),
                    _raw(th, 2 * HID, [[1, n]]),
                    1.0, _raw(th, 3 * HID, [[1, n]]), op0=ALU.add, op1=ALU.mult,
                )
                c_new = state.tile([HID, NBH], F32, tag="c")
                nc.vector.scalar_tensor_tensor(
                    B3(c_new, lo, hi + 1),
                    _raw(p, 0, [[m, BL], [1, m]]),
                    0.5, _raw(r_t, 0, [[m, BL], [1, m]]),
                    op0=ALU.mult, op1=ALU.add,
                )

                tanc = tmp.tile([HID, NBH], F32, tag="tanc")
                nc.scalar.activation(
                    _raw(tanc, 0, [[m, BL], [1, m]]),
                    B3(c_new, lo, hi + 1), AF.Tanh, scale=0.5,
                )
                h_new = state.tile([HID, NBH], F32, tag="h")
                nc.vector.scalar_tensor_tensor(
                    B3(h_new, lo, hi + 1),
                    _raw(th, 4 * HID, [[m, BL], [1, m]]),
                    1.0, _raw(tanc, 0, [[m, BL], [1, m]]),
                    op0=ALU.add, op1=ALU.mult,
                )

                # seed the virgin row t+1 for the next step (both blocks, h+c)
                if t + 1 <= H - 1:
                    for bb in range(BL):
                        nc.gpsimd.tensor_copy(
                            out=_raw(h_new, bb * H + t + 1, [[1, 1]]),
                            in_=hv[:, t:t + 1],
                        )
                        nc.gpsimd.tensor_copy(
                            out=_raw(c_new, bb * H + t + 1, [[1, 1]]),
                            in_=cv[:, t:t + 1],
                        )

                # write H2 into res along the diagonal w = t - r (gpsimd)
                res_ap = _raw(res, (W - 1) * lo + t, [[H * W, BL], [W - 1, m]])
                nc.gpsimd.tensor_copy(out=res_ap, in_=B3(h_new, lo, hi + 1))

                h_cur = h_new
                c_cur = c_new

                # Late-scan interleave: once an 8-row block's diagonal cells
                # are all written (t = 8j+70), add its residual (fp32r
                # matmul; feed-forward so reduced precision is safe) and DMA
                # it out.
                if t >= 70 and (t - 70) % 8 == 0 and (t - 70) // 8 < 8:
                    j = (t - 70) // 8
                    for b in range(BL):
                        cols = slice(b * H * W + 512 * j, b * H * W + 512 * j + 512)
                        rp = pbankO()
                        nc.tensor.matmul(
                            rp, wres, xres[:, cols], start=True, stop=True
                        )
                        nc.vector.tensor_add(res[:, cols], res[:, cols], rp)
                        nc.sync.dma_start(out=out_d[:, cols], in_=res[:, cols])

    nc.finalize()
    return nc


_NC_CACHE = {}


def _get_nc():
    if "nc" not in _NC_CACHE:
        _NC_CACHE["nc"] = build_program()
    return _NC_CACHE["nc"]


def _round_fp32r(x):
    """RNE to fp32r (11 explicit mantissa bits), matching PE input rounding."""
    u = np.ascontiguousarray(x, np.float32).view(np.uint32).astype(np.uint64)
    drop = 12
    u2 = u + ((1 << (drop - 1)) - 1) + ((u >> drop) & 1)
    u2 &= ~np.uint64((1 << drop) - 1)
    return u2.astype(np.uint32).view(np.float32)


def _virgin_tables(w_ss, b_is, b_ss):
    """fp64 recurrence for the shared zero-input state v_t, t = 0..62.

    Rows r > t all hold v_t (their whole dependency cone saw zero x), so the
    device only computes the live band and copies v_t into row t+1.
    Returns device-convention tables (2h, 2c), [HID, 63]."""
    w0 = np.asarray(w_ss, np.float64)[:, :, 0]
    w1 = np.asarray(w_ss, np.float64)[:, :, 1]
    bb = np.asarray(b_is, np.float64) + np.asarray(b_ss, np.float64)
    wsum = w0 + w1
    h = np.zeros(HID)
    c = np.zeros(HID)
    hv = np.zeros((HID, H - 1), np.float64)
    cv = np.zeros((HID, H - 1), np.float64)
    for t in range(H - 1):
        z = bb + wsum @ h
        o, fl, fu, i, g = np.split(z, 5)
        sig = lambda v: 1.0 / (1.0 + np.exp(-v))
        o, fl, fu, i = sig(o), sig(fl), sig(fu), sig(i)
        c = fl * c + fu * c + i * np.tanh(g)
        h = o * np.tanh(c)
        hv[:, t] = 2.0 * h
        cv[:, t] = 2.0 * c
    return hv.astype(np.float32), cv.astype(np.float32)


def _prep_inputs(x, w_is, b_is, w_ss, b_ss, w_res, b_res):
    x = np.asarray(x, np.float32)
    # band-packed skewed x: col _BASE[t] + b*m + (r - lo) = x[b, :, r, t - r]
    xs = x.reshape(NCORES, BL, C, H, W)
    xsk = np.zeros((NCORES, C + 1, XC), np.float32)
    xsk[:, C, :] = 1.0  # ones row: adds the per-gate bias via the matmul
    for t in range(SW):
        lo, hi, m = _LO[t], _HI[t], _M[t]
        rows = np.arange(lo, hi + 1)
        blk = xs[:, :, :, rows, t - rows]          # [cores, BL, C, m]
        blk = blk.transpose(0, 2, 1, 3)            # [cores, C, BL, m]
        xsk[:, :C, _BASE[t]:_BASE[t + 1]] = blk.reshape(NCORES, C, BL * m)

    xres = np.asarray(x).reshape(NCORES, BL, C, H, W).transpose(0, 2, 1, 3, 4)
    xres = xres.reshape(NCORES, C, NRES)
    xres = np.concatenate([xres, np.ones((NCORES, 1, NRES), np.float32)], axis=1)
    xres = _round_fp32r(xres).reshape(NCORES, C + 1, NRES)

    # gate scaling: chunks 0..3 (o, f_left, f_up, i) are sigmoid gates,
    # computed via tanh(z/2) -> pre-halve their weights and biases.
    gs = np.ones((5 * HID,), np.float32)
    gs[0:4 * HID] = 0.5

    # wtap[i, tap*640 + o] = w_ss[o, i, tap] * gs[o] * 0.5
    # (extra 0.5: the kernel's h state holds 2h)
    wtap = np.asarray(w_ss, np.float32).transpose(1, 2, 0) * (0.5 * gs)[None, None, :]
    wtap = np.ascontiguousarray(wtap.reshape(HID, 2 * 5 * HID), np.float32)
    # wis row C (the ones row's partner) carries the combined gate bias
    bvec = (np.asarray(b_is, np.float32) + np.asarray(b_ss, np.float32)) * gs
    wis = np.ascontiguousarray(
        np.concatenate(
            [np.asarray(w_is, np.float32).T * gs[None, :], bvec[None, :]], axis=0
        ),
        np.float32,
    )
    # x2: the device residual tile accumulates 2*(residual + sum h); the
    # host halves the final output.
    wres = 2.0 * np.concatenate(
        [np.asarray(w_res, np.float32).T, np.asarray(b_res, np.float32)[None, :]],
        axis=0,
    ).astype(np.float32)
    wres = _round_fp32r(wres).reshape(C + 1, HID)

    hv, cv = _virgin_tables(w_ss, b_is, b_ss)

    in_maps = []
    for c in range(NCORES):
        in_maps.append({
            "xsk": np.ascontiguousarray(xsk[c]),
            "xres": np.ascontiguousarray(xres[c]),
            "wtap": wtap,
            "wis": wis,
            "wres": wres,
            "hv": hv,
            "cv": cv,
        })
    return in_maps


def kernel(x, w_is, b_is, w_ss, b_ss, w_res, b_res, _trace=False):
    nc = _get_nc()
    in_maps = _prep_inputs(x, w_is, b_is, w_ss, b_ss, w_res, b_res)
    r = run_bass_kernel_spmd(nc, in_maps, list(range(NCORES)), trace=_trace)
    outs = [r.results[c]["out"] for c in range(NCORES)]
    out = np.stack(outs, 0).reshape(NCORES, HID, BL, H, W)
    out = out.transpose(0, 2, 1, 3, 4).reshape(B, HID, H, W)
    return np.ascontiguousarray(out * np.float32(0.5))
